# revision 8
# baseline (speedup 1.0000x reference)
"""Trainium2 Bass kernel for nn_BindingSiteGNN (2-layer GATv2 GNN).

Strategy (8 NeuronCores, node-partitioned):
  - 2500 nodes per core; local node l -> (block t = l % 20, partition p = l // 20).
  - Dense transforms (h @ W) run on the owning core; the source-side
    transformed features (xl | att.xl) are AllGathered as one fp16 table per
    layer; edge tiles gather source rows by index (indirect DMA).
  - Per 128-edge tile: one-hot matrices built on-device from dst indices drive
    PSUM matmuls for xr[dst] expansion, segment-sum scatter (numerator),
    softmax denominator, and self-loop edge_attr means. Softmax runs without
    max-subtraction (logits are tiny) and the division is hoisted out of the
    edge loop (numer/denom per destination).
  - leaky_relu(u) dot att is computed exactly as 0.2*(att.u) + 0.8*(att.relu(u)):
    the linear term via tiny N=4 matmuls against pre-contracted weights, the
    relu term via ScalarE Relu + VectorE multiply/reduce.
"""
import sys
sys.path.insert(0, '/opt/trn_rl_repo')
import numpy as np

N, E_REF = 20000, 150000
NCORES = 8
NC = N // NCORES            # 2500
TPB = 20                    # dst blocks per core
PPB = NC // TPB             # 125 real partitions per block
NCPAD = TPB * 128           # 2560
IN_DIM, AA_EMB, NUM_AA = 5, 32, 20
H1, HEADS, HID = 512, 4, 128
F0 = 128
TOT_IN = IN_DIM + AA_EMB    # 37
GB = 4                      # tiles per gather DMA


def _wrap16(idx):
    n = len(idx)
    out = np.zeros((16, n // 16), np.int16)
    out[np.arange(n) % 16, np.arange(n) // 16] = idx
    return out


def prep_all(inputs):
    """Shard + build all per-core host arrays and the SPMD tile schedule."""
    x = np.ascontiguousarray(np.asarray(inputs['x'], np.float32))
    ei = np.asarray(inputs['edge_index'], np.int64)
    ea = np.asarray(inputs['edge_attr'], np.float32)
    res = np.asarray(inputs['residue_type'], np.int64)

    src, dst = ei[0], ei[1]
    core_of = dst // NC
    percore = []
    counts = np.zeros((NCORES, TPB), np.int64)
    for c in range(NCORES):
        sel = np.nonzero(core_of == c)[0]
        es, eda, dl = src[sel], ea[sel], dst[sel] - c * NC
        t, p = dl % TPB, dl // TPB
        order = np.lexsort((p, t))
        es, eda, t, p, dl = es[order], eda[order], t[order], p[order], dl[order]
        percore.append((es, eda, t, p, dl))
        counts[c] = np.bincount(t, minlength=TPB)
    ntile_real = np.maximum((counts + 127) // 128, 1).max(axis=0)  # per block

    # SPMD-uniform schedule: per block t, ntile_real[t] real tiles + 1 self
    sched = []          # (t, is_self)
    for tt in range(TPB):
        sched += [(tt, False)] * int(ntile_real[tt])
        sched.append((tt, True))
    NT = len(sched)

    gcol = lambda g: (g // NC) * NCPAD + ((g % NC) % TPB) * 128 + (g % NC) // TPB

    cores = []
    for c in range(NCORES):
        es, eda, t, p, dl = percore[c]
        gidx = np.zeros((NT, 128), np.int64)
        drel = np.full((NT, 128), 127, np.int64)
        eat = np.zeros((NT, 128, 2), np.float32)
        it = 0
        for tt in range(TPB):
            m = t == tt
            ss, pp, ee = es[m], p[m], eda[m]
            nreal = len(ss)
            for k in range(int(ntile_real[tt])):
                lo, hi = k * 128, min(k * 128 + 128, nreal)
                if hi > lo:
                    nn_ = hi - lo
                    gidx[it, :nn_] = gcol(ss[lo:hi])
                    drel[it, :nn_] = pp[lo:hi]
                    eat[it, :nn_] = ee[lo:hi]
                it += 1
            # self tile
            gidx[it] = c * NCPAD + tt * 128 + np.arange(128)
            drel[it] = np.arange(128)
            it += 1
        assert it == NT

        deg = np.bincount(dl, minlength=NC).astype(np.float32)
        inv_deg = np.zeros((128, TPB), np.float32)
        ll = np.arange(NC)
        inv_deg[ll // TPB, ll % TPB] = 1.0 / np.maximum(deg, 1.0)

        x_own = np.zeros((128, TPB, IN_DIM), np.float32)
        x_own[ll // TPB, ll % TPB] = x[c * NC + ll]

        roh = np.zeros((NUM_AA, NCPAD), np.float16)
        rows = (ll % TPB) * 128 + ll // TPB
        roh[res[c * NC + ll], rows] = 1.0

        cores.append(dict(
            idx32=np.ascontiguousarray(gidx.T.astype(np.int32)),       # [128, NT]
            drel=np.ascontiguousarray(drel.T.astype(np.float32)),      # [128, NT]
            eaT=np.ascontiguousarray(
                np.transpose(eat, (2, 0, 1)).astype(np.float16)),      # [2, NT, 128]
            ea_em=np.ascontiguousarray(eat.transpose(1, 0, 2).astype(np.float16)),
            inv_deg=inv_deg, x_own=x_own, res_onehotT=roh,
        ))

    # shared weights / constants
    f16 = lambda a: np.ascontiguousarray(np.asarray(a, np.float16))
    f32c = lambda a: np.ascontiguousarray(np.asarray(a, np.float32))
    W1l_pad = np.zeros((F0, H1), np.float32)
    W1r_pad = np.zeros((F0, H1), np.float32)
    W1l_pad[:TOT_IN] = inputs['W1l']
    W1r_pad[:TOT_IN] = inputs['W1r']
    att1 = np.asarray(inputs['att1'], np.float32).reshape(1, HEADS * HID)
    att2 = np.asarray(inputs['att2'], np.float32).reshape(1, HID)
    shared = dict(
        W1l_pad=f16(W1l_pad), W1r_pad=f16(W1r_pad), We1=f16(inputs['W1e']),
        att1_02=f16(np.broadcast_to(0.2 * att1, (128, H1))),
        att1_08=f16(np.broadcast_to(0.8 * att1, (128, H1))),
        att2_02=f16(np.broadcast_to(0.2 * att2, (128, HID))),
        att2_08=f16(np.broadcast_to(0.8 * att2, (128, HID))),
        W2l_ch=f16(np.asarray(inputs['W2l']).reshape(4, 128, HID).transpose(1, 0, 2)),
        W2r_ch=f16(np.asarray(inputs['W2r']).reshape(4, 128, HID).transpose(1, 0, 2)),
        We2=f16(inputs['W2e']), Wfc=f16(inputs['Wfc']),
        b1_rep=f32c(np.broadcast_to(np.asarray(inputs['b1'], np.float32), (128, H1))),
        b2_rep=f32c(np.broadcast_to(np.asarray(inputs['b2'], np.float32), (128, HID))),
        bfc_rep=f32c(np.broadcast_to(np.asarray(inputs['bfc'], np.float32), (128, 2))),
        aa_emb=f16(inputs['aa_emb']),
        iota_rep=f32c(np.broadcast_to(np.arange(128, dtype=np.float32), (128, 128))),
        ident16=f16(np.eye(128)),
    )
    return sched, cores, shared


def build_program(sched):
    import concourse.bass as bass
    import concourse.bacc as bacc
    import concourse.mybir as mybir
    import concourse.tile as tile

    f32, f16, i32 = mybir.dt.float32, mybir.dt.float16, mybir.dt.int32
    AF = mybir.ActivationFunctionType
    OP = mybir.AluOpType
    NT = len(sched)

    nc = bacc.Bacc("TRN2", target_bir_lowering=False, debug=False,
                   num_devices=NCORES)

    # ---- I/O ----
    EI = lambda n, s, d: nc.dram_tensor(n, s, d, kind="ExternalInput")
    t_idx32 = EI("idx32", [128, NT], i32)
    t_drel = EI("drel", [128, NT], f32)
    t_eaT = EI("eaT", [2, NT, 128], f16)
    t_eaem = EI("ea_em", [128, NT, 2], f16)
    t_invdeg = EI("inv_deg", [128, TPB], f32)
    t_xown = EI("x_own", [128, TPB, IN_DIM], f32)
    t_roh = EI("res_onehotT", [NUM_AA, NCPAD], f16)
    t_W1l = EI("W1l_pad", [F0, H1], f16)
    t_W1r = EI("W1r_pad", [F0, H1], f16)
    t_We1 = EI("We1", [2, H1], f16)
    t_a102 = EI("att1_02", [128, H1], f16)
    t_a108 = EI("att1_08", [128, H1], f16)
    t_a202 = EI("att2_02", [128, HID], f16)
    t_a208 = EI("att2_08", [128, HID], f16)
    t_W2l = EI("W2l_ch", [128, 4, HID], f16)
    t_W2r = EI("W2r_ch", [128, 4, HID], f16)
    t_We2 = EI("We2", [2, HID], f16)
    t_Wfc = EI("Wfc", [128, 2], f16)
    t_b1 = EI("b1_rep", [128, H1], f32)
    t_b2 = EI("b2_rep", [128, HID], f32)
    t_bfc = EI("bfc_rep", [128, 2], f32)
    t_aa = EI("aa_emb", [NUM_AA, AA_EMB], f16)
    t_iota = EI("iota_rep", [128, 128], f32)
    t_id16 = EI("ident16", [128, 128], f16)
    t_out = nc.dram_tensor("out", [NCPAD, 2], f32, kind="ExternalOutput")
    import os
    DBG = os.environ.get("GNN_DEBUG", "0") == "1"
    if DBG:
        d_xl1own = nc.dram_tensor("d_xl1own", [NCPAD, H1 + 4], f16, kind="ExternalOutput")
        d_xl1full = nc.dram_tensor("d_xl1full", [NCPAD, H1 + 4], f16, kind="ExternalOutput")
        d_xr1 = nc.dram_tensor("d_xr1", [128, TPB * H1], f16, kind="ExternalOutput")
        d_h1T = nc.dram_tensor("d_h1T", [128, 4 * TPB * 128], f16, kind="ExternalOutput")
        d_loop = nc.dram_tensor("d_loop", [2, TPB * 128], f16, kind="ExternalOutput")
        d_xl2full = nc.dram_tensor("d_xl2full", [NCPAD, HID + 4], f16, kind="ExternalOutput")
        d_g0 = nc.dram_tensor("d_g0", [128, H1 + 4], f16, kind="ExternalOutput")
        d_alpha0 = nc.dram_tensor("d_alpha0", [128, 4], f32, kind="ExternalOutput")
        d_m0 = nc.dram_tensor("d_m0", [128, H1], f16, kind="ExternalOutput")

    # internal DRAM
    xl1_own = nc.dram_tensor("xl1_own", [NCPAD, H1 + 4], f16)
    xl1_full = nc.dram_tensor("xl1_full", [NCORES * NCPAD, H1 + 4], f16,
                              addr_space="Shared")
    xl2_own = nc.dram_tensor("xl2_own", [NCPAD, HID + 4], f16)
    xl2_full = nc.dram_tensor("xl2_full", [NCORES * NCPAD, HID + 4], f16,
                              addr_space="Shared")

    RG = [list(range(NCORES))]

    with tile.TileContext(nc) as tc:
        import contextlib
        ctx = contextlib.ExitStack()
        with ctx:
            per = ctx.enter_context(tc.tile_pool(name="persist", bufs=1))
            wrk = ctx.enter_context(tc.tile_pool(name="work", bufs=3))
            big = ctx.enter_context(tc.tile_pool(name="bigwork", bufs=2))
            ps_u = ctx.enter_context(tc.tile_pool(name="ps_u", bufs=2, space="PSUM"))
            ps_lin = ctx.enter_context(tc.tile_pool(name="ps_lin", bufs=2, space="PSUM"))
            ps_sm = ctx.enter_context(tc.tile_pool(name="ps_sm", bufs=2, space="PSUM"))
            ps_num = ctx.enter_context(tc.tile_pool(name="ps_num", bufs=1, space="PSUM"))
            ps_acc = ctx.enter_context(tc.tile_pool(name="ps_acc", bufs=1, space="PSUM"))

            # ---- persistent loads ----
            def load(t, shape, dtype):
                s = per.tile(shape, dtype, tag=f"ld_{t.name}")
                nc.sync.dma_start(s[...], t[...])
                return s

            idx32 = load(t_idx32, [128, NT], i32)
            drel = load(t_drel, [128, NT], f32)
            eaT = load(t_eaT, [2, NT, 128], f16)
            eaem = load(t_eaem, [128, NT, 2], f16)
            invdeg = load(t_invdeg, [128, TPB], f32)
            roh = load(t_roh, [NUM_AA, NCPAD], f16)
            W1l = load(t_W1l, [F0, H1], f16)
            W1r = load(t_W1r, [F0, H1], f16)
            We1 = load(t_We1, [2, H1], f16)
            a102 = load(t_a102, [128, H1], f16)
            a108 = load(t_a108, [128, H1], f16)
            a202 = load(t_a202, [128, HID], f16)
            a208 = load(t_a208, [128, HID], f16)
            W2l = load(t_W2l, [128, 4, HID], f16)
            W2r = load(t_W2r, [128, 4, HID], f16)
            We2 = load(t_We2, [2, HID], f16)
            Wfc = load(t_Wfc, [128, 2], f16)
            b1r = load(t_b1, [128, H1], f32)
            b2r = load(t_b2, [128, HID], f32)
            bfcr = load(t_bfc, [128, 2], f32)
            aa = load(t_aa, [NUM_AA, AA_EMB], f16)
            iota = load(t_iota, [128, 128], f32)
            id16 = load(t_id16, [128, 128], f16)

            # persistent state
            h0T = per.tile([128, TPB, F0], f16)          # feature-major h0
            xr1 = per.tile([128, TPB, H1], f16)
            xr1a = per.tile([128, TPB, 4], f16)
            h1T = per.tile([128, 4, TPB, 128], f16)
            xr2 = per.tile([128, TPB, HID], f16)
            xr2a = per.tile([128, TPB, 4], f16)          # col 0 used
            loop_save = per.tile([2, TPB, 128], f16)     # scaled loop_ea^T

            # ---- contracted attention weights (device) ----
            def att_contract(dst_tile, W_sb, att02_sb, nheads, width, parts=128):
                scr = wrk.tile([128, H1], f16, tag="attscr")
                nc.vector.tensor_tensor(
                    out=scr[:parts, :width], in0=W_sb, in1=att02_sb[:parts, :width],
                    op=OP.mult)
                red = wrk.tile([128, 4], f32, tag="attred")
                nc.vector.tensor_reduce(
                    out=red[:parts, :nheads],
                    in_=scr[:parts, :width].rearrange("p (h c) -> p h c", h=nheads),
                    axis=mybir.AxisListType.X, op=OP.add)
                nc.vector.tensor_copy(dst_tile, red[:parts, :nheads])

            Wl_a = per.tile([128, 4], f16)
            Wr_a = per.tile([128, 4], f16)
            We1_a = per.tile([2, 4], f16)
            att_contract(Wl_a[...], W1l[...], a102, 4, H1)
            att_contract(Wr_a[...], W1r[...], a102, 4, H1)
            att_contract(We1_a[...], We1[...], a102, 4, H1, parts=2)
            W2l_a = per.tile([128, 4], f16)              # col k = chunk k
            W2r_a = per.tile([128, 4], f16)
            We2_a = per.tile([2, 4], f16)                # col 0
            for k in range(4):
                att_contract(W2l_a[:, k:k + 1], W2l[:, k, :], a202, 1, HID)
                att_contract(W2r_a[:, k:k + 1], W2r[:, k, :], a202, 1, HID)
            att_contract(We2_a[:, 0:1], We2[...], a202, 1, HID, parts=2)

            # ---- phase 0: h0 assembly + h0T ----
            h0 = per.tile([128, TPB, F0], f16)
            nc.gpsimd.memset(h0[...], 0.0)
            xst = wrk.tile([128, TPB, IN_DIM], f32, tag="xst")
            nc.sync.dma_start(xst[...], t_xown[...])
            nc.scalar.copy(h0[:, :, 0:IN_DIM], xst[...])
            for tt in range(TPB):
                pe_emb = ps_sm.tile([128, AA_EMB], f32, space="PSUM", tag="psT")
                nc.tensor.matmul(pe_emb[...], lhsT=roh[:, tt * 128:(tt + 1) * 128],
                                 rhs=aa[...], start=True, stop=True)
                nc.scalar.copy(h0[:, tt, IN_DIM:TOT_IN], pe_emb[...])
            for tt in range(TPB):
                pT = ps_sm.tile([128, 128], f16, space="PSUM", tag="psT")
                nc.tensor.transpose(pT[...], h0[:, tt, :], id16[...])
                nc.scalar.copy(h0T[:, tt, :], pT[...])

            # ---- phase 1 dense: xl1(+att) -> DRAM; xr1(+att) -> SBUF ----
            for tt in range(TPB):
                lhs = h0T[:, tt, :]
                p_xl = ps_u.tile([128, H1], f32, space="PSUM", tag="pu")
                nc.tensor.matmul(p_xl[...], lhsT=lhs, rhs=W1l[...], start=True, stop=True)
                p_la = ps_lin.tile([128, 4], f32, space="PSUM", tag="plin")
                nc.tensor.matmul(p_la[...], lhsT=lhs, rhs=Wl_a[...], start=True, stop=True)
                xl_sb = big.tile([128, H1 + 4], f16, tag="xlsb")
                nc.scalar.copy(xl_sb[:, 0:H1], p_xl[...])
                nc.scalar.copy(xl_sb[:, H1:H1 + 4], p_la[...])
                nc.sync.dma_start(xl1_own[tt * 128:(tt + 1) * 128, :], xl_sb[...])
                p_xr = ps_u.tile([128, H1], f32, space="PSUM", tag="pu")
                nc.tensor.matmul(p_xr[...], lhsT=lhs, rhs=W1r[...], start=True, stop=True)
                nc.scalar.copy(xr1[:, tt, :], p_xr[...])
                p_ra = ps_lin.tile([128, 4], f32, space="PSUM", tag="plin")
                nc.tensor.matmul(p_ra[...], lhsT=lhs, rhs=Wr_a[...], start=True, stop=True)
                nc.scalar.copy(xr1a[:, tt, :], p_ra[...])

            nc.gpsimd.collective_compute(
                "AllGather", mybir.AluOpType.bypass, replica_groups=RG,
                ins=[xl1_own[:, :].opt()], outs=[xl1_full[:, :].opt()])
            if DBG:
                nc.sync.dma_start(d_xl1own[:, :], xl1_own[:, :])
                nc.sync.dma_start(d_xl1full[:, :], xl1_full[0:NCPAD, :])
                nc.sync.dma_start(d_xr1[:, :], xr1[...].rearrange("p t f -> p (t f)"))

            # ---- edge phase (shared for both layers) ----
            def edge_layer(layer, on_block):
                if layer == 1:
                    F, NH = H1, HEADS
                    feat_full, FW = xl1_full, H1 + 4
                    xr_sb, xra_sb = xr1, xr1a
                    We_sb, Wea_sb = We1, We1_a
                    a08 = a108
                else:
                    F, NH = HID, 1
                    feat_full, FW = xl2_full, HID + 4
                    xr_sb, xra_sb = xr2, xr2a
                    We_sb, Wea_sb = We2, We2_a
                    a08 = a208

                numer = None
                bacc_t = None
                g_ext = None
                for it, (tt, is_self) in enumerate(sched):
                    first = it == 0 or sched[it - 1][0] != tt
                    last = is_self
                    g_ext = big.tile([128, FW], f16, tag="gext")
                    nc.gpsimd.indirect_dma_start(
                        out=g_ext[...], out_offset=None,
                        in_=feat_full[:, :],
                        in_offset=bass.IndirectOffsetOnAxis(
                            ap=idx32[:, it:it + 1], axis=0))
                    if first:
                        numer = ps_num.tile([128, F], f32, space="PSUM", tag="num")
                        bacc_t = ps_acc.tile([128, 8], f32, space="PSUM", tag="acc")
                    # one-hot (edge-major) and its transpose
                    oh = wrk.tile([128, 128], f16, tag="oh")
                    nc.vector.tensor_scalar(
                        out=oh[...], in0=iota[...], scalar1=drel[:, it:it + 1],
                        scalar2=None, op0=OP.is_equal)
                    pT = ps_sm.tile([128, 128], f16, space="PSUM", tag="psT")
                    nc.tensor.transpose(pT[...], oh[...], id16[...])
                    ohT = wrk.tile([128, 128], f16, tag="ohT")
                    nc.scalar.copy(ohT[...], pT[...])
                    # self-tile edge attr = scaled loop_ea
                    if is_self:
                        if layer == 1:
                            lsc = wrk.tile([128, 2], f16, tag="lsc")
                            nc.vector.tensor_scalar(
                                out=lsc[...], in0=bacc_t[:, 4:6],
                                scalar1=invdeg[:, tt:tt + 1], scalar2=None,
                                op0=OP.mult)
                            pL = ps_sm.tile([2, 128], f16, space="PSUM", tag="psT")
                            nc.tensor.transpose(pL[...], lsc[...], id16[...])
                            nc.scalar.copy(loop_save[:, tt, :], pL[...])
                        ea_lhsT = loop_save[:, tt, :]
                    else:
                        ea_lhsT = eaT[:, it, :]
                    # u = G + onehot@xr + ea@We   (PSUM)
                    p_u = ps_u.tile([128, F], f32, space="PSUM", tag="pu")
                    nc.tensor.matmul(p_u[...], lhsT=ohT[...], rhs=xr_sb[:, tt, :],
                                     start=True, stop=False)
                    nc.tensor.matmul(p_u[...], lhsT=ea_lhsT, rhs=We_sb[...],
                                     start=False, stop=False, skip_group_check=True)
                    nc.tensor.matmul(p_u[...], lhsT=id16[...],
                                     rhs=g_ext[:, 0:F],
                                     start=False, stop=True, skip_group_check=True)
                    # lin = 0.2 * att.u  (PSUM)
                    p_lin = ps_lin.tile([128, 4], f32, space="PSUM", tag="plin")
                    nc.tensor.matmul(p_lin[:, 0:NH], lhsT=ohT[...],
                                     rhs=xra_sb[:, tt, 0:NH], start=True, stop=False)
                    nc.tensor.matmul(p_lin[:, 0:NH], lhsT=ea_lhsT,
                                     rhs=Wea_sb[:, 0:NH], start=False, stop=False,
                                     skip_group_check=True)
                    nc.tensor.matmul(p_lin[:, 0:NH], lhsT=id16[...],
                                     rhs=g_ext[:, F:F + NH], start=False,
                                     stop=True, skip_group_check=True)
                    # m = relu(u); alpha = sum(m*0.8att) + lin; p = exp(alpha)
                    m = big.tile([128, F], f16, tag="mrelu")
                    nc.scalar.activation(m[...], p_u[...], AF.Relu)
                    if DBG and layer == 1 and it == 0:
                        nc.sync.dma_start(d_g0[:, :], g_ext[...])
                        nc.sync.dma_start(d_m0[:, :], m[...])
                    tp = big.tile([128, F], f16, tag="tprod")
                    nc.vector.tensor_tensor(out=tp[...], in0=m[...], in1=a08[:, 0:F],
                                            op=OP.mult)
                    red = wrk.tile([128, 4], f32, tag="red")
                    nc.vector.tensor_reduce(
                        out=red[:, 0:NH],
                        in_=tp[...].rearrange("p (h c) -> p h c", h=NH),
                        axis=mybir.AxisListType.X, op=OP.add)
                    alpha = wrk.tile([128, 4], f32, tag="alpha")
                    nc.vector.tensor_tensor(out=alpha[:, 0:NH], in0=red[:, 0:NH],
                                            in1=p_lin[:, 0:NH], op=OP.add)
                    pv = wrk.tile([128, 4], f32, tag="pv")
                    nc.scalar.activation(pv[:, 0:NH], alpha[:, 0:NH], AF.Exp)
                    if DBG and layer == 1 and it == 0:
                        nc.sync.dma_start(d_alpha0[:, :], alpha[...])
                    pv16 = wrk.tile([128, 4], f16, tag="pv16")
                    nc.scalar.copy(pv16[:, 0:NH], pv[:, 0:NH])
                    # scatter: numer += phot_h.T @ G_h ; denom += onehot.T @ p
                    phot = wrk.tile([128, 4, 128], f16, tag="phot")
                    for h in range(NH):
                        nc.vector.tensor_scalar(
                            out=phot[:, h, :], in0=iota[...],
                            scalar1=drel[:, it:it + 1], scalar2=pv[:, h:h + 1],
                            op0=OP.is_equal, op1=OP.mult)
                        nc.tensor.matmul(
                            numer[:, h * 128:(h + 1) * 128], lhsT=phot[:, h, :],
                            rhs=g_ext[:, h * 128:(h + 1) * 128],
                            start=first and h == 0, stop=last and h == NH - 1,
                            skip_group_check=True)
                    nc.tensor.matmul(bacc_t[:, 0:NH], lhsT=oh[...],
                                     rhs=pv16[:, 0:NH], start=first, stop=last,
                                     skip_group_check=True)
                    if not is_self and layer == 1:
                        nc.tensor.matmul(bacc_t[:, 4:6], lhsT=oh[...],
                                         rhs=eaem[:, it, :], start=False,
                                         stop=False, skip_group_check=True)
                    # ---- block evacuation ----
                    if is_self:
                        rec = wrk.tile([128, 4], f32, tag="rec")
                        nc.vector.reciprocal(rec[:, 0:NH], bacc_t[:, 0:NH])
                        o1 = big.tile([128, F], f32, tag="oblk")
                        nc.vector.tensor_tensor(
                            out=o1[...].rearrange("p (h c) -> p h c", h=NH),
                            in0=numer[...].rearrange("p (h c) -> p h c", h=NH),
                            in1=rec[:, 0:NH, None].to_broadcast([128, NH, F // NH]),
                            op=OP.mult)
                        on_block(tt, o1)

            def elu_block(o1, brep, F, hdst):
                """h = elu(o1 + b) (fp16) written to hdst [128, F]."""
                ob = big.tile([128, F], f32, tag="ob")
                nc.vector.tensor_tensor(out=ob[...], in0=o1[...], in1=brep[:, 0:F],
                                        op=OP.add)
                q = big.tile([128, F], f32, tag="q")
                nc.vector.tensor_scalar(out=q[...], in0=ob[...], scalar1=0.0,
                                        scalar2=None, op0=OP.min)
                eq = big.tile([128, F], f32, tag="eq")
                nc.scalar.activation(eq[...], q[...], AF.Exp)
                r = big.tile([128, F], f32, tag="r")
                nc.vector.tensor_scalar(out=r[...], in0=ob[...], scalar1=0.0,
                                        scalar2=None, op0=OP.max)
                s = big.tile([128, F], f32, tag="s")
                nc.vector.tensor_tensor(out=s[...], in0=r[...], in1=eq[...], op=OP.add)
                nc.vector.tensor_scalar(out=hdst, in0=s[...], scalar1=-1.0,
                                        scalar2=None, op0=OP.add)

            # ===== layer 1 =====
            def on_block1(tt, o1):
                h1b = big.tile([128, H1], f16, tag="h1b")
                elu_block(o1, b1r, H1, h1b[...])
                for k in range(4):
                    pT = ps_sm.tile([128, 128], f16, space="PSUM", tag="psT")
                    nc.tensor.transpose(pT[...], h1b[:, k * 128:(k + 1) * 128],
                                        id16[...])
                    nc.scalar.copy(h1T[:, k, tt, :], pT[...])
            edge_layer(1, on_block1)

            # dense 2
            for tt in range(TPB):
                p_xl = ps_u.tile([128, HID], f32, space="PSUM", tag="pu")
                p_la = ps_lin.tile([128, 4], f32, space="PSUM", tag="plin")
                p_xr = ps_u.tile([128, HID], f32, space="PSUM", tag="pu")
                p_ra = ps_lin.tile([128, 4], f32, space="PSUM", tag="plin")
                for k in range(4):
                    lhs = h1T[:, k, tt, :]
                    st = k == 0
                    sp = k == 3
                    nc.tensor.matmul(p_xl[...], lhsT=lhs, rhs=W2l[:, k, :],
                                     start=st, stop=sp, skip_group_check=True)
                    nc.tensor.matmul(p_la[:, 0:1], lhsT=lhs, rhs=W2l_a[:, k:k + 1],
                                     start=st, stop=sp, skip_group_check=True)
                    nc.tensor.matmul(p_xr[...], lhsT=lhs, rhs=W2r[:, k, :],
                                     start=st, stop=sp, skip_group_check=True)
                    nc.tensor.matmul(p_ra[:, 0:1], lhsT=lhs, rhs=W2r_a[:, k:k + 1],
                                     start=st, stop=sp, skip_group_check=True)
                xl_sb = wrk.tile([128, HID + 4], f16, tag="xl2sb")
                nc.scalar.copy(xl_sb[:, 0:HID], p_xl[...])
                nc.scalar.copy(xl_sb[:, HID:HID + 1], p_la[:, 0:1])
                nc.gpsimd.memset(xl_sb[:, HID + 1:HID + 4], 0.0)
                nc.sync.dma_start(xl2_own[tt * 128:(tt + 1) * 128, :], xl_sb[...])
                nc.scalar.copy(xr2[:, tt, :], p_xr[...])
                nc.scalar.copy(xr2a[:, tt, 0:1], p_ra[:, 0:1])

            nc.gpsimd.collective_compute(
                "AllGather", mybir.AluOpType.bypass, replica_groups=RG,
                ins=[xl2_own[:, :].opt()], outs=[xl2_full[:, :].opt()])
            if DBG:
                nc.sync.dma_start(d_h1T[:, :], h1T[...].rearrange("p k t f -> p (k t f)"))
                nc.sync.dma_start(d_loop[:, :], loop_save[...].rearrange("p t f -> p (t f)"))
                nc.sync.dma_start(d_xl2full[:, :], xl2_full[0:NCPAD, :])

            # ===== layer 2 =====
            def on_block2(tt, o1):
                h2b = wrk.tile([128, HID], f16, tag="h2b")
                elu_block(o1, b2r, HID, h2b[...])
                pT = ps_sm.tile([128, 128], f16, space="PSUM", tag="psT")
                nc.tensor.transpose(pT[...], h2b[...], id16[...])
                h2T = wrk.tile([128, 128], f16, tag="h2T")
                nc.scalar.copy(h2T[...], pT[...])
                p_fc = ps_lin.tile([128, 4], f32, space="PSUM", tag="plin")
                nc.tensor.matmul(p_fc[:, 0:2], lhsT=h2T[...], rhs=Wfc[...],
                                 start=True, stop=True, skip_group_check=True)
                o_sb = wrk.tile([128, 2], f32, tag="osb")
                nc.vector.tensor_tensor(out=o_sb[...], in0=p_fc[:, 0:2],
                                        in1=bfcr[:, 0:2], op=OP.add)
                nc.sync.dma_start(t_out[tt * 128:(tt + 1) * 128, :], o_sb[...])
            edge_layer(2, on_block2)

    nc.compile()
    return nc


_CACHE = {}


def kernel(**inputs):
    from concourse.bass_utils import run_bass_kernel_spmd

    sched, cores, shared = prep_all(inputs)
    key = tuple(sched)
    if key not in _CACHE:
        _CACHE[key] = build_program(sched)
    nc = _CACHE[key]

    in_maps = []
    for c in range(NCORES):
        m = dict(shared)
        m.update(cores[c])
        m = {k: v for k, v in m.items()}
        m['idx32'] = cores[c]['idx32']
        in_maps.append(m)
    res = run_bass_kernel_spmd(nc, in_maps, core_ids=list(range(NCORES)))

    out = np.zeros((N, 2), np.float32)
    ll = np.arange(NC)
    rows = (ll % TPB) * 128 + ll // TPB
    for c in range(NCORES):
        out[c * NC:(c + 1) * NC] = res.results[c]["out"][rows]
    return out


# revision 13
# speedup vs baseline: 718.8902x; 718.8902x over previous
"""Trainium2 Bass kernel for nn_BindingSiteGNN (2-layer GATv2 GNN).

Strategy (8 NeuronCores, node-partitioned):
  - 2500 nodes per core; local node l -> (block t = l % 20, partition p = l // 20).
  - Dense transforms (h @ W) run on the owning core; the source-side
    transformed features (xl | att.xl) are AllGathered as one fp16 table per
    layer; edge tiles gather source rows by index (indirect DMA).
  - Per 128-edge tile: one-hot matrices built on-device from dst indices drive
    PSUM matmuls for xr[dst] expansion, segment-sum scatter (numerator),
    softmax denominator, and self-loop edge_attr means. Softmax runs without
    max-subtraction (logits are tiny) and the division is hoisted out of the
    edge loop (numer/denom per destination).
  - leaky_relu(u) dot att is computed exactly as 0.2*(att.u) + 0.8*(att.relu(u)):
    the linear term via tiny N=4 matmuls against pre-contracted weights, the
    relu term via ScalarE Relu + VectorE multiply/reduce.
"""
import sys
sys.path.insert(0, '/opt/trn_rl_repo')
import numpy as np

N, E_REF = 20000, 150000
NCORES = 8
NC = N // NCORES            # 2500
TPB = 20                    # dst blocks per core
PPB = NC // TPB             # 125 real partitions per block
NCPAD = TPB * 128           # 2560
IN_DIM, AA_EMB, NUM_AA = 5, 32, 20
H1, HEADS, HID = 512, 4, 128
F0 = 128
TOT_IN = IN_DIM + AA_EMB    # 37
GB = 4                      # tiles per gather DMA


def _wrap16(idx):
    n = len(idx)
    out = np.zeros((16, n // 16), np.int16)
    out[np.arange(n) % 16, np.arange(n) // 16] = idx
    return out


def prep_all(inputs):
    """Shard + build all per-core host arrays and the SPMD tile schedule."""
    x = np.ascontiguousarray(np.asarray(inputs['x'], np.float32))
    ei = np.asarray(inputs['edge_index'], np.int64)
    ea = np.asarray(inputs['edge_attr'], np.float32)
    res = np.asarray(inputs['residue_type'], np.int64)

    src, dst = ei[0], ei[1]
    core_of = dst // NC
    percore = []
    counts = np.zeros((NCORES, TPB), np.int64)
    for c in range(NCORES):
        sel = np.nonzero(core_of == c)[0]
        es, eda, dl = src[sel], ea[sel], dst[sel] - c * NC
        t, p = dl % TPB, dl // TPB
        order = np.lexsort((p, t))
        es, eda, t, p, dl = es[order], eda[order], t[order], p[order], dl[order]
        percore.append((es, eda, t, p, dl))
        counts[c] = np.bincount(t, minlength=TPB)
    ntile_real = np.maximum((counts + 127) // 128, 1).max(axis=0)  # per block

    # SPMD-uniform schedule: per block t, ntile_real[t] real tiles + 1 self
    sched = []          # (t, is_self)
    for tt in range(TPB):
        sched += [(tt, False)] * int(ntile_real[tt])
        sched.append((tt, True))
    NT = len(sched)

    gcol = lambda g: (g // NC) * NCPAD + ((g % NC) % TPB) * 128 + (g % NC) // TPB

    cores = []
    for c in range(NCORES):
        es, eda, t, p, dl = percore[c]
        gidx = np.zeros((NT, 128), np.int64)
        drel = np.full((NT, 128), 127, np.int64)
        eat = np.zeros((NT, 128, 2), np.float32)
        it = 0
        for tt in range(TPB):
            m = t == tt
            ss, pp, ee = es[m], p[m], eda[m]
            nreal = len(ss)
            for k in range(int(ntile_real[tt])):
                lo, hi = k * 128, min(k * 128 + 128, nreal)
                if hi > lo:
                    nn_ = hi - lo
                    gidx[it, :nn_] = gcol(ss[lo:hi])
                    drel[it, :nn_] = pp[lo:hi]
                    eat[it, :nn_] = ee[lo:hi]
                it += 1
            # self tile
            gidx[it] = c * NCPAD + tt * 128 + np.arange(128)
            drel[it] = np.arange(128)
            it += 1
        assert it == NT

        deg = np.bincount(dl, minlength=NC).astype(np.float32)
        inv_deg = np.zeros((128, TPB), np.float32)
        ll = np.arange(NC)
        inv_deg[ll // TPB, ll % TPB] = 1.0 / np.maximum(deg, 1.0)

        x_own = np.zeros((128, TPB, IN_DIM), np.float32)
        x_own[ll // TPB, ll % TPB] = x[c * NC + ll]

        roh = np.zeros((NUM_AA, NCPAD), np.float16)
        rows = (ll % TPB) * 128 + ll // TPB
        roh[res[c * NC + ll], rows] = 1.0

        cores.append(dict(
            idx32=np.ascontiguousarray(gidx.T.astype(np.int32)),       # [128, NT]
            drel=np.ascontiguousarray(drel.T.astype(np.float32)),      # [128, NT]
            eaT=np.ascontiguousarray(
                np.transpose(eat, (2, 0, 1)).astype(np.float16)),      # [2, NT, 128]
            ea_em=np.ascontiguousarray(eat.transpose(1, 0, 2).astype(np.float16)),
            inv_deg=inv_deg, x_own=x_own, res_onehotT=roh,
        ))

    # shared weights / constants
    f16 = lambda a: np.ascontiguousarray(np.asarray(a, np.float16))
    f32c = lambda a: np.ascontiguousarray(np.asarray(a, np.float32))
    W1l_pad = np.zeros((F0, H1), np.float32)
    W1r_pad = np.zeros((F0, H1), np.float32)
    W1l_pad[:TOT_IN] = inputs['W1l']
    W1r_pad[:TOT_IN] = inputs['W1r']
    att1 = np.asarray(inputs['att1'], np.float32).reshape(1, HEADS * HID)
    att2 = np.asarray(inputs['att2'], np.float32).reshape(1, HID)
    shared = dict(
        W1l_pad=f16(W1l_pad), W1r_pad=f16(W1r_pad), We1=f16(inputs['W1e']),
        att1_02=f16(np.broadcast_to(0.2 * att1, (128, H1))),
        att1_08=f16(np.broadcast_to(0.8 * att1, (128, H1))),
        att2_02=f16(np.broadcast_to(0.2 * att2, (128, HID))),
        att2_08=f16(np.broadcast_to(0.8 * att2, (128, HID))),
        W2l_ch=f16(np.asarray(inputs['W2l']).reshape(4, 128, HID).transpose(1, 0, 2)),
        W2r_ch=f16(np.asarray(inputs['W2r']).reshape(4, 128, HID).transpose(1, 0, 2)),
        We2=f16(inputs['W2e']), Wfc=f16(inputs['Wfc']),
        b1_rep=f32c(np.broadcast_to(np.asarray(inputs['b1'], np.float32), (128, H1))),
        b2_rep=f32c(np.broadcast_to(np.asarray(inputs['b2'], np.float32), (128, HID))),
        bfc_rep=f32c(np.broadcast_to(np.asarray(inputs['bfc'], np.float32), (128, 2))),
        aa_emb=f16(inputs['aa_emb']),
        iota_rep=f32c(np.broadcast_to(np.arange(128, dtype=np.float32), (128, 128))),
        ident16=f16(np.eye(128)),
    )
    return sched, cores, shared


def build_program(sched):
    import concourse.bass as bass
    import concourse.bacc as bacc
    import concourse.mybir as mybir
    import concourse.tile as tile

    f32, f16, i32 = mybir.dt.float32, mybir.dt.float16, mybir.dt.int32
    AF = mybir.ActivationFunctionType
    OP = mybir.AluOpType
    NT = len(sched)

    nc = bacc.Bacc("TRN2", target_bir_lowering=False, debug=False,
                   num_devices=NCORES)

    # ---- I/O ----
    EI = lambda n, s, d: nc.dram_tensor(n, s, d, kind="ExternalInput")
    t_idx32 = EI("idx32", [128, NT], i32)
    t_drel = EI("drel", [128, NT], f32)
    t_eaT = EI("eaT", [2, NT, 128], f16)
    t_eaem = EI("ea_em", [128, NT, 2], f16)
    t_invdeg = EI("inv_deg", [128, TPB], f32)
    t_xown = EI("x_own", [128, TPB, IN_DIM], f32)
    t_roh = EI("res_onehotT", [NUM_AA, NCPAD], f16)
    t_W1l = EI("W1l_pad", [F0, H1], f16)
    t_W1r = EI("W1r_pad", [F0, H1], f16)
    t_We1 = EI("We1", [2, H1], f16)
    t_a102 = EI("att1_02", [128, H1], f16)
    t_a108 = EI("att1_08", [128, H1], f16)
    t_a202 = EI("att2_02", [128, HID], f16)
    t_a208 = EI("att2_08", [128, HID], f16)
    t_W2l = EI("W2l_ch", [128, 4, HID], f16)
    t_W2r = EI("W2r_ch", [128, 4, HID], f16)
    t_We2 = EI("We2", [2, HID], f16)
    t_Wfc = EI("Wfc", [128, 2], f16)
    t_b1 = EI("b1_rep", [128, H1], f32)
    t_b2 = EI("b2_rep", [128, HID], f32)
    t_bfc = EI("bfc_rep", [128, 2], f32)
    t_aa = EI("aa_emb", [NUM_AA, AA_EMB], f16)
    t_iota = EI("iota_rep", [128, 128], f32)
    t_id16 = EI("ident16", [128, 128], f16)
    t_out = nc.dram_tensor("out", [NCPAD, 2], f32, kind="ExternalOutput")
    import os
    DBG = os.environ.get("GNN_DEBUG", "0") == "1"
    if DBG:
        d_xl1own = nc.dram_tensor("d_xl1own", [NCPAD, H1 + 4], f16, kind="ExternalOutput")
        d_xl1full = nc.dram_tensor("d_xl1full", [NCPAD, H1 + 4], f16, kind="ExternalOutput")
        d_xr1 = nc.dram_tensor("d_xr1", [128, TPB * H1], f16, kind="ExternalOutput")
        d_h1T = nc.dram_tensor("d_h1T", [128, 4 * TPB * 128], f16, kind="ExternalOutput")
        d_loop = nc.dram_tensor("d_loop", [2, TPB * 128], f16, kind="ExternalOutput")
        d_xl2full = nc.dram_tensor("d_xl2full", [NCPAD, HID + 4], f16, kind="ExternalOutput")
        d_g0 = nc.dram_tensor("d_g0", [128, H1 + 4], f16, kind="ExternalOutput")
        d_alpha0 = nc.dram_tensor("d_alpha0", [128, 4], f32, kind="ExternalOutput")
        d_m0 = nc.dram_tensor("d_m0", [128, H1], f16, kind="ExternalOutput")

    # internal DRAM
    xl1_own = nc.dram_tensor("xl1_own", [NCPAD, H1 + 4], f16)
    xl1_full = nc.dram_tensor("xl1_full", [NCORES * NCPAD, H1 + 4], f16,
                              addr_space="Shared")
    xl2_own = nc.dram_tensor("xl2_own", [NCPAD, HID + 4], f16)
    xl2_full = nc.dram_tensor("xl2_full", [NCORES * NCPAD, HID + 4], f16,
                              addr_space="Shared")

    RG = [list(range(NCORES))]

    with tile.TileContext(nc) as tc:
        import contextlib
        ctx = contextlib.ExitStack()
        with ctx:
            per = ctx.enter_context(tc.tile_pool(name="persist", bufs=1))
            wrk = ctx.enter_context(tc.tile_pool(name="work", bufs=3))
            big = ctx.enter_context(tc.tile_pool(name="bigwork", bufs=2))
            ps_u = ctx.enter_context(tc.tile_pool(name="ps_u", bufs=2, space="PSUM"))
            ps_lin = ctx.enter_context(tc.tile_pool(name="ps_lin", bufs=2, space="PSUM"))
            ps_sm = ctx.enter_context(tc.tile_pool(name="ps_sm", bufs=2, space="PSUM"))
            ps_num = ctx.enter_context(tc.tile_pool(name="ps_num", bufs=1, space="PSUM"))
            ps_acc = ctx.enter_context(tc.tile_pool(name="ps_acc", bufs=1, space="PSUM"))

            # ---- persistent loads ----
            def load(t, shape, dtype):
                s = per.tile(shape, dtype, tag=f"ld_{t.name}")
                nc.sync.dma_start(s[...], t[...])
                return s

            idx32 = load(t_idx32, [128, NT], i32)
            drel = load(t_drel, [128, NT], f32)
            eaT = load(t_eaT, [2, NT, 128], f16)
            eaem = load(t_eaem, [128, NT, 2], f16)
            invdeg = load(t_invdeg, [128, TPB], f32)
            roh = load(t_roh, [NUM_AA, NCPAD], f16)
            W1l = load(t_W1l, [F0, H1], f16)
            W1r = load(t_W1r, [F0, H1], f16)
            We1 = load(t_We1, [2, H1], f16)
            a102 = load(t_a102, [128, H1], f16)
            a108 = load(t_a108, [128, H1], f16)
            a202 = load(t_a202, [128, HID], f16)
            a208 = load(t_a208, [128, HID], f16)
            W2l = load(t_W2l, [128, 4, HID], f16)
            W2r = load(t_W2r, [128, 4, HID], f16)
            We2 = load(t_We2, [2, HID], f16)
            Wfc = load(t_Wfc, [128, 2], f16)
            b1r = load(t_b1, [128, H1], f32)
            b2r = load(t_b2, [128, HID], f32)
            bfcr = load(t_bfc, [128, 2], f32)
            aa = load(t_aa, [NUM_AA, AA_EMB], f16)
            iota = load(t_iota, [128, 128], f32)
            id16 = load(t_id16, [128, 128], f16)

            # persistent state
            h0T = per.tile([128, TPB, F0], f16)          # feature-major h0
            xr1 = per.tile([128, TPB, H1], f16)
            xr1a = per.tile([128, TPB, 4], f16)
            h1T = per.tile([128, 4, TPB, 128], f16)
            xr2 = per.tile([128, TPB, HID], f16)
            xr2a = per.tile([128, TPB, 4], f16)          # col 0 used
            loop_save = per.tile([2, TPB, 128], f16)     # scaled loop_ea^T

            # ---- contracted attention weights (device) ----
            def att_contract(dst_tile, W_sb, att02_sb, nheads, width, parts=128):
                scr = wrk.tile([128, H1], f16, tag="attscr")
                nc.vector.tensor_tensor(
                    out=scr[:parts, :width], in0=W_sb, in1=att02_sb[:parts, :width],
                    op=OP.mult)
                red = wrk.tile([128, 4], f32, tag="attred")
                nc.vector.tensor_reduce(
                    out=red[:parts, :nheads],
                    in_=scr[:parts, :width].rearrange("p (h c) -> p h c", h=nheads),
                    axis=mybir.AxisListType.X, op=OP.add)
                nc.vector.tensor_copy(dst_tile, red[:parts, :nheads])

            Wl_a = per.tile([128, 4], f16)
            Wr_a = per.tile([128, 4], f16)
            We1_a = per.tile([2, 4], f16)
            att_contract(Wl_a[...], W1l[...], a102, 4, H1)
            att_contract(Wr_a[...], W1r[...], a102, 4, H1)
            att_contract(We1_a[...], We1[...], a102, 4, H1, parts=2)
            W2l_a = per.tile([128, 4], f16)              # col k = chunk k
            W2r_a = per.tile([128, 4], f16)
            We2_a = per.tile([2, 4], f16)                # col 0
            for k in range(4):
                att_contract(W2l_a[:, k:k + 1], W2l[:, k, :], a202, 1, HID)
                att_contract(W2r_a[:, k:k + 1], W2r[:, k, :], a202, 1, HID)
            att_contract(We2_a[:, 0:1], We2[...], a202, 1, HID, parts=2)

            # ---- phase 0: h0 assembly + h0T ----
            h0 = per.tile([128, TPB, F0], f16)
            nc.gpsimd.memset(h0[...], 0.0)
            xst = wrk.tile([128, TPB, IN_DIM], f32, tag="xst")
            nc.sync.dma_start(xst[...], t_xown[...])
            nc.scalar.copy(h0[:, :, 0:IN_DIM], xst[...])
            for tt in range(TPB):
                pe_emb = ps_sm.tile([128, AA_EMB], f32, space="PSUM", tag="psT")
                nc.tensor.matmul(pe_emb[...], lhsT=roh[:, tt * 128:(tt + 1) * 128],
                                 rhs=aa[...], start=True, stop=True)
                nc.scalar.copy(h0[:, tt, IN_DIM:TOT_IN], pe_emb[...])
            for tt in range(TPB):
                pT = ps_sm.tile([128, 128], f16, space="PSUM", tag="psT")
                nc.tensor.transpose(pT[...], h0[:, tt, :], id16[...])
                nc.scalar.copy(h0T[:, tt, :], pT[...])

            # ---- phase 1 dense: xl1(+att) -> DRAM; xr1(+att) -> SBUF ----
            for tt in range(TPB):
                lhs = h0T[:, tt, :]
                p_xl = ps_u.tile([128, H1], f32, space="PSUM", tag="pu")
                nc.tensor.matmul(p_xl[...], lhsT=lhs, rhs=W1l[...], start=True, stop=True)
                p_la = ps_lin.tile([128, 4], f32, space="PSUM", tag="plin")
                nc.tensor.matmul(p_la[...], lhsT=lhs, rhs=Wl_a[...], start=True, stop=True)
                xl_sb = big.tile([128, H1 + 4], f16, tag="xlsb")
                nc.scalar.copy(xl_sb[:, 0:H1], p_xl[...])
                nc.scalar.copy(xl_sb[:, H1:H1 + 4], p_la[...])
                nc.sync.dma_start(xl1_own[tt * 128:(tt + 1) * 128, :], xl_sb[...])
                p_xr = ps_u.tile([128, H1], f32, space="PSUM", tag="pu")
                nc.tensor.matmul(p_xr[...], lhsT=lhs, rhs=W1r[...], start=True, stop=True)
                nc.scalar.copy(xr1[:, tt, :], p_xr[...])
                p_ra = ps_lin.tile([128, 4], f32, space="PSUM", tag="plin")
                nc.tensor.matmul(p_ra[...], lhsT=lhs, rhs=Wr_a[...], start=True, stop=True)
                nc.scalar.copy(xr1a[:, tt, :], p_ra[...])

            nc.gpsimd.collective_compute(
                "AllGather", mybir.AluOpType.bypass, replica_groups=RG,
                ins=[xl1_own[:, :].opt()], outs=[xl1_full[:, :].opt()])
            if DBG:
                nc.sync.dma_start(d_xl1own[:, :], xl1_own[:, :])
                nc.sync.dma_start(d_xl1full[:, :], xl1_full[0:NCPAD, :])
                nc.sync.dma_start(d_xr1[:, :], xr1[...].rearrange("p t f -> p (t f)"))

            # ---- edge phase (shared for both layers) ----
            def edge_layer(layer, on_block):
                if layer == 1:
                    F, NH = H1, HEADS
                    feat_full, FW = xl1_full, H1 + 4
                    xr_sb, xra_sb = xr1, xr1a
                    We_sb, Wea_sb = We1, We1_a
                    a08 = a108
                else:
                    F, NH = HID, 1
                    feat_full, FW = xl2_full, HID + 4
                    xr_sb, xra_sb = xr2, xr2a
                    We_sb, Wea_sb = We2, We2_a
                    a08 = a208

                numer = None
                bacc_t = None
                g_ext = None
                for it, (tt, is_self) in enumerate(sched):
                    first = it == 0 or sched[it - 1][0] != tt
                    last = is_self
                    g_ext = big.tile([128, FW], f16, tag="gext")
                    nc.gpsimd.indirect_dma_start(
                        out=g_ext[...], out_offset=None,
                        in_=feat_full[:, :],
                        in_offset=bass.IndirectOffsetOnAxis(
                            ap=idx32[:, it:it + 1], axis=0))
                    if first:
                        numer = ps_num.tile([128, F], f32, space="PSUM", tag="num")
                        bacc_t = ps_acc.tile([128, 8], f32, space="PSUM", tag="acc")
                    # one-hot (edge-major) and its transpose
                    oh = wrk.tile([128, 128], f16, tag="oh")
                    nc.vector.tensor_scalar(
                        out=oh[...], in0=iota[...], scalar1=drel[:, it:it + 1],
                        scalar2=None, op0=OP.is_equal)
                    pT = ps_sm.tile([128, 128], f16, space="PSUM", tag="psT")
                    nc.tensor.transpose(pT[...], oh[...], id16[...])
                    ohT = wrk.tile([128, 128], f16, tag="ohT")
                    nc.scalar.copy(ohT[...], pT[...])
                    # self-tile edge attr = scaled loop_ea
                    if is_self:
                        if layer == 1:
                            lsc = wrk.tile([128, 2], f16, tag="lsc")
                            nc.vector.tensor_scalar(
                                out=lsc[...], in0=bacc_t[:, 4:6],
                                scalar1=invdeg[:, tt:tt + 1], scalar2=None,
                                op0=OP.mult)
                            pL = ps_sm.tile([2, 128], f16, space="PSUM", tag="psT")
                            nc.tensor.transpose(pL[...], lsc[...], id16[...])
                            nc.scalar.copy(loop_save[:, tt, :], pL[...])
                        ea_lhsT = loop_save[:, tt, :]
                    else:
                        ea_lhsT = eaT[:, it, :]
                    # u = G + onehot@xr + ea@We   (PSUM)
                    p_u = ps_u.tile([128, F], f32, space="PSUM", tag="pu")
                    nc.tensor.matmul(p_u[...], lhsT=ohT[...], rhs=xr_sb[:, tt, :],
                                     start=True, stop=False)
                    nc.tensor.matmul(p_u[...], lhsT=ea_lhsT, rhs=We_sb[...],
                                     start=False, stop=False, skip_group_check=True)
                    nc.tensor.matmul(p_u[...], lhsT=id16[...],
                                     rhs=g_ext[:, 0:F],
                                     start=False, stop=True, skip_group_check=True)
                    # lin = 0.2 * att.u  (PSUM)
                    p_lin = ps_lin.tile([128, 4], f32, space="PSUM", tag="plin")
                    nc.tensor.matmul(p_lin[:, 0:NH], lhsT=ohT[...],
                                     rhs=xra_sb[:, tt, 0:NH], start=True, stop=False)
                    nc.tensor.matmul(p_lin[:, 0:NH], lhsT=ea_lhsT,
                                     rhs=Wea_sb[:, 0:NH], start=False, stop=False,
                                     skip_group_check=True)
                    nc.tensor.matmul(p_lin[:, 0:NH], lhsT=id16[...],
                                     rhs=g_ext[:, F:F + NH], start=False,
                                     stop=True, skip_group_check=True)
                    # m = relu(u); alpha = sum(m*0.8att) + lin; p = exp(alpha)
                    m = big.tile([128, F], f16, tag="mrelu")
                    nc.scalar.activation(m[...], p_u[...], AF.Relu)
                    if DBG and layer == 1 and it == 0:
                        nc.sync.dma_start(d_g0[:, :], g_ext[...])
                        nc.sync.dma_start(d_m0[:, :], m[...])
                    tp = big.tile([128, F], f16, tag="tprod")
                    nc.vector.tensor_tensor(out=tp[...], in0=m[...], in1=a08[:, 0:F],
                                            op=OP.mult)
                    red = wrk.tile([128, 4], f32, tag="red")
                    nc.vector.tensor_reduce(
                        out=red[:, 0:NH],
                        in_=tp[...].rearrange("p (h c) -> p h c", h=NH),
                        axis=mybir.AxisListType.X, op=OP.add)
                    alpha = wrk.tile([128, 4], f32, tag="alpha")
                    nc.vector.tensor_tensor(out=alpha[:, 0:NH], in0=red[:, 0:NH],
                                            in1=p_lin[:, 0:NH], op=OP.add)
                    pv = wrk.tile([128, 4], f32, tag="pv")
                    nc.scalar.activation(pv[:, 0:NH], alpha[:, 0:NH], AF.Exp)
                    if DBG and layer == 1 and it == 0:
                        nc.sync.dma_start(d_alpha0[:, :], alpha[...])
                    pv16 = wrk.tile([128, 4], f16, tag="pv16")
                    nc.scalar.copy(pv16[:, 0:NH], pv[:, 0:NH])
                    # scatter: numer += phot_h.T @ G_h ; denom += onehot.T @ p
                    phot = wrk.tile([128, 4, 128], f16, tag="phot")
                    for h in range(NH):
                        nc.vector.tensor_scalar(
                            out=phot[:, h, :], in0=iota[...],
                            scalar1=drel[:, it:it + 1], scalar2=pv[:, h:h + 1],
                            op0=OP.is_equal, op1=OP.mult)
                        nc.tensor.matmul(
                            numer[:, h * 128:(h + 1) * 128], lhsT=phot[:, h, :],
                            rhs=g_ext[:, h * 128:(h + 1) * 128],
                            start=first and h == 0, stop=last and h == NH - 1,
                            skip_group_check=True)
                    nc.tensor.matmul(bacc_t[:, 0:NH], lhsT=oh[...],
                                     rhs=pv16[:, 0:NH], start=first, stop=last,
                                     skip_group_check=True)
                    if not is_self and layer == 1:
                        nc.tensor.matmul(bacc_t[:, 4:6], lhsT=oh[...],
                                         rhs=eaem[:, it, :], start=False,
                                         stop=False, skip_group_check=True)
                    # ---- block evacuation ----
                    if is_self:
                        rec = wrk.tile([128, 4], f32, tag="rec")
                        nc.vector.reciprocal(rec[:, 0:NH], bacc_t[:, 0:NH])
                        o1 = big.tile([128, F], f32, tag="oblk")
                        nc.vector.tensor_tensor(
                            out=o1[...].rearrange("p (h c) -> p h c", h=NH),
                            in0=numer[...].rearrange("p (h c) -> p h c", h=NH),
                            in1=rec[:, 0:NH, None].to_broadcast([128, NH, F // NH]),
                            op=OP.mult)
                        on_block(tt, o1)

            def elu_block(o1, brep, F, hdst):
                """h = elu(o1 + b) (fp16) written to hdst [128, F]."""
                ob = big.tile([128, F], f32, tag="ob")
                nc.vector.tensor_tensor(out=ob[...], in0=o1[...], in1=brep[:, 0:F],
                                        op=OP.add)
                q = big.tile([128, F], f32, tag="q")
                nc.vector.tensor_scalar(out=q[...], in0=ob[...], scalar1=0.0,
                                        scalar2=None, op0=OP.min)
                eq = big.tile([128, F], f32, tag="eq")
                nc.scalar.activation(eq[...], q[...], AF.Exp)
                r = big.tile([128, F], f32, tag="r")
                nc.vector.tensor_scalar(out=r[...], in0=ob[...], scalar1=0.0,
                                        scalar2=None, op0=OP.max)
                s = big.tile([128, F], f32, tag="s")
                nc.vector.tensor_tensor(out=s[...], in0=r[...], in1=eq[...], op=OP.add)
                nc.vector.tensor_scalar(out=hdst, in0=s[...], scalar1=-1.0,
                                        scalar2=None, op0=OP.add)

            # ===== layer 1 =====
            def on_block1(tt, o1):
                h1b = big.tile([128, H1], f16, tag="h1b")
                elu_block(o1, b1r, H1, h1b[...])
                for k in range(4):
                    pT = ps_sm.tile([128, 128], f16, space="PSUM", tag="psT")
                    nc.tensor.transpose(pT[...], h1b[:, k * 128:(k + 1) * 128],
                                        id16[...])
                    nc.scalar.copy(h1T[:, k, tt, :], pT[...])
            edge_layer(1, on_block1)

            # dense 2
            for tt in range(TPB):
                p_xl = ps_u.tile([128, HID], f32, space="PSUM", tag="pu")
                p_la = ps_lin.tile([128, 4], f32, space="PSUM", tag="plin")
                p_xr = ps_u.tile([128, HID], f32, space="PSUM", tag="pu")
                p_ra = ps_lin.tile([128, 4], f32, space="PSUM", tag="plin")
                for k in range(4):
                    lhs = h1T[:, k, tt, :]
                    st = k == 0
                    sp = k == 3
                    nc.tensor.matmul(p_xl[...], lhsT=lhs, rhs=W2l[:, k, :],
                                     start=st, stop=sp, skip_group_check=True)
                    nc.tensor.matmul(p_la[:, 0:1], lhsT=lhs, rhs=W2l_a[:, k:k + 1],
                                     start=st, stop=sp, skip_group_check=True)
                    nc.tensor.matmul(p_xr[...], lhsT=lhs, rhs=W2r[:, k, :],
                                     start=st, stop=sp, skip_group_check=True)
                    nc.tensor.matmul(p_ra[:, 0:1], lhsT=lhs, rhs=W2r_a[:, k:k + 1],
                                     start=st, stop=sp, skip_group_check=True)
                xl_sb = wrk.tile([128, HID + 4], f16, tag="xl2sb")
                nc.scalar.copy(xl_sb[:, 0:HID], p_xl[...])
                nc.scalar.copy(xl_sb[:, HID:HID + 1], p_la[:, 0:1])
                nc.gpsimd.memset(xl_sb[:, HID + 1:HID + 4], 0.0)
                nc.sync.dma_start(xl2_own[tt * 128:(tt + 1) * 128, :], xl_sb[...])
                nc.scalar.copy(xr2[:, tt, :], p_xr[...])
                nc.scalar.copy(xr2a[:, tt, 0:1], p_ra[:, 0:1])

            nc.gpsimd.collective_compute(
                "AllGather", mybir.AluOpType.bypass, replica_groups=RG,
                ins=[xl2_own[:, :].opt()], outs=[xl2_full[:, :].opt()])
            if DBG:
                nc.sync.dma_start(d_h1T[:, :], h1T[...].rearrange("p k t f -> p (k t f)"))
                nc.sync.dma_start(d_loop[:, :], loop_save[...].rearrange("p t f -> p (t f)"))
                nc.sync.dma_start(d_xl2full[:, :], xl2_full[0:NCPAD, :])

            # ===== layer 2 =====
            def on_block2(tt, o1):
                h2b = wrk.tile([128, HID], f16, tag="h2b")
                elu_block(o1, b2r, HID, h2b[...])
                pT = ps_sm.tile([128, 128], f16, space="PSUM", tag="psT")
                nc.tensor.transpose(pT[...], h2b[...], id16[...])
                h2T = wrk.tile([128, 128], f16, tag="h2T")
                nc.scalar.copy(h2T[...], pT[...])
                p_fc = ps_lin.tile([128, 4], f32, space="PSUM", tag="plin")
                nc.tensor.matmul(p_fc[:, 0:2], lhsT=h2T[...], rhs=Wfc[...],
                                 start=True, stop=True, skip_group_check=True)
                o_sb = wrk.tile([128, 2], f32, tag="osb")
                nc.vector.tensor_tensor(out=o_sb[...], in0=p_fc[:, 0:2],
                                        in1=bfcr[:, 0:2], op=OP.add)
                nc.sync.dma_start(t_out[tt * 128:(tt + 1) * 128, :], o_sb[...])
            edge_layer(2, on_block2)

    nc.compile()
    return nc


_CACHE = {}


def kernel(**inputs):
    from concourse.bass_utils import run_bass_kernel_spmd

    sched, cores, shared = prep_all(inputs)
    key = tuple(sched)
    if key not in _CACHE:
        _CACHE[key] = build_program(sched)
    nc = _CACHE[key]

    in_maps = []
    for c in range(NCORES):
        m = dict(shared)
        m.update(cores[c])
        m = {k: v for k, v in m.items()}
        m['idx32'] = cores[c]['idx32']
        in_maps.append(m)
    res = run_bass_kernel_spmd(nc, in_maps, core_ids=list(range(NCORES)))

    out = np.zeros((N, 2), np.float32)
    ll = np.arange(NC)
    rows = (ll % TPB) * 128 + ll // TPB
    for c in range(NCORES):
        out[c * NC:(c + 1) * NC] = res.results[c]["out"][rows]
    return out
